# revision 26
# baseline (speedup 1.0000x reference)
import sys

sys.path.insert(0, "/opt/trn_rl_repo")
from contextlib import ExitStack

import numpy as np

import concourse.bass as bass
import concourse.mybir as mybir
import concourse.tile as tile
from concourse import bacc

# ---- problem constants (hardcoded; must match reference.py) ----
B, C, IMG = 2, 96, 256
WS = 2
NS = IMG // WS          # 128 patches per side
N = NS * NS             # 16384 tokens
TD = C * WS * WS        # 384 token dim
H = 6                   # heads
D = TD // H             # 64 head dim
W1 = 128                # one-sided window
G = 50                  # global tokens
NCORES = 8
SPLITS = 4              # sequence splits per batch
QLEN = N // SPLITS      # 4096 queries per core
NCH = QLEN // W1        # 32 query chunks per core
HALO = W1
NTOK = QLEN + 2 * HALO  # 4352 tokens incl halo
KCH = NCH + 2           # 34 key chunks incl halo
GPOS = np.linspace(0, N - 1, G).astype(np.int32)
E = D + 1               # head dim + denominator column

_cache = {}


def _build_program(reps=1):
    f32 = mybir.dt.float32
    f16 = mybir.dt.float16
    bf16 = mybir.dt.bfloat16
    AF = mybir.ActivationFunctionType
    nc = bacc.Bacc("TRN2", target_bir_lowering=False, debug=False,
                   num_devices=NCORES)

    # ---- DRAM I/O ----
    tokT_d = nc.dram_tensor("tokT", [TD, NTOK], f16, kind="ExternalInput")
    wnames = ["wq", "wk", "wv", "wkg", "wvg", "wqg"]
    # host pre-transposes weights to [128, 3, TD] (1 descriptor/partition)
    w_d = {nm: nc.dram_tensor(nm, [128, 3, TD], f16, kind="ExternalInput")
           for nm in wnames}
    bnames = ["bq", "bk", "bkg", "bqg"]
    # packed small constants: biases [128, 4*3] f32; masks+tokgT f16
    ball_d = nc.dram_tensor("ball", [128, 12], f32, kind="ExternalInput")
    small_d = nc.dram_tensor("small", [128, 3 * 512 + 3 * G], f16,
                             kind="ExternalInput")
    # outputs: q-major attention out (64 numerator cols + denom) per head pair
    out_d = nc.dram_tensor("out_t", [H // 2, NCH // 2, W1, 4 * E], bf16,
                           kind="ExternalOutput")
    og_d = nc.dram_tensor("og_part", [H, G, E], f32, kind="ExternalOutput")

    with tile.TileContext(nc) as tc, ExitStack() as ctx:
        const = ctx.enter_context(tc.tile_pool(name="const", bufs=1))
        tokp = ctx.enter_context(tc.tile_pool(name="tokp", bufs=1))
        vp = ctx.enter_context(tc.tile_pool(name="vp", bufs=1))
        pairp = ctx.enter_context(tc.tile_pool(name="pairp", bufs=2))
        kgap = ctx.enter_context(tc.tile_pool(name="kgap", bufs=3))
        pp = ctx.enter_context(tc.tile_pool(name="pp", bufs=8))
        outp = ctx.enter_context(tc.tile_pool(name="outp", bufs=4))
        psA = ctx.enter_context(tc.tile_pool(name="psA", bufs=2, space="PSUM"))
        psS = ctx.enter_context(tc.tile_pool(name="psS", bufs=2, space="PSUM"))
        psO = ctx.enter_context(tc.tile_pool(name="psO", bufs=2, space="PSUM"))

        # ---- constants into SBUF (DMA order tuned for fast rampup) ----
        ball_sb = const.tile([128, 12], f32, name="ball_sb")
        nc.scalar.dma_start(out=ball_sb, in_=ball_d[:, :])
        b_sb = {nm: ball_sb[:, 3 * i:3 * i + 3]
                for i, nm in enumerate(bnames)}
        small_sb = const.tile([128, 3 * 512 + 3 * G], f16, name="small_sb")
        nc.scalar.dma_start(out=small_sb, in_=small_d[:, :])
        m_sb = {nm: small_sb[:, 512 * i:512 * i + 512]
                for i, nm in enumerate(["m_std", "m_first", "m_last"])}
        tokgT_sb = small_sb[:, 3 * 512:].rearrange("p (m g) -> p m g", m=3)
        # tokens: 6 half-DMAs (per mi, low cols first) release deps early;
        # weight DMAs interleaved so the first proj units start ~6us in
        w_sb = {}

        def w_dma(nm, eng):
            t = const.tile([128, 3, TD], f16, name=f"{nm}_sb")
            eng.dma_start(out=t, in_=w_d[nm][:, :, :])
            w_sb[nm] = t

        # tokens + wk on the SP HWDGE queue; other weights/constants load
        # in parallel on the Activation HWDGE queue
        tokT_sb = tokp.tile([128, 3, NTOK], f16, name="tokT_sb")
        HTOK = NTOK // 2
        w_dma("wk", nc.sync)
        w_dma("wq", nc.scalar)
        w_dma("wv", nc.scalar)
        for mi in range(3):
            nc.sync.dma_start(out=tokT_sb[:, mi, 0:HTOK],
                              in_=tokT_d[mi * 128:(mi + 1) * 128, 0:HTOK])
        w_dma("wkg", nc.scalar)
        for mi in range(3):
            nc.sync.dma_start(out=tokT_sb[:, mi, HTOK:NTOK],
                              in_=tokT_d[mi * 128:(mi + 1) * 128, HTOK:NTOK])
        w_dma("wvg", nc.scalar)
        w_dma("wqg", nc.scalar)

        # ---- compute body (repeatable for benchmarking) ----
        if reps > 1:
            loop_ctx = tc.For_i(0, reps, 1)
            loop_ctx.__enter__()
        for _rep in range(1):
            # global-token projections: kgT (Wk) + vg_aug (Wv) up front;
            # qgT (Wqg) is deferred into band1 fills (wqg is the last DMA)
            qgT_sb = vp.tile([128, 3, G], f16, name="qgT_sb", tag="qgT")
            kgT_sb = vp.tile([128, 3, 128], f16, name="kgT_sb", tag="kgT")
            vg_aug = vp.tile([128, H, E], f16, name="vg_aug", tag="vgaug")
            nc.vector.memset(kgT_sb, 0.0)
            nc.vector.memset(vg_aug, 0.0)
            for mi in range(3):
                ms = slice(mi * 128, (mi + 1) * 128)
                ps_k = psA.tile([128, 512], f32, name="ps_gk", tag="pj")
                for kj in range(3):
                    nc.tensor.matmul(ps_k[:, 0:G], lhsT=w_sb["wk"][:, kj, ms],
                                     rhs=tokgT_sb[:, kj, :],
                                     start=kj == 0, stop=kj == 2)
                nc.vector.tensor_scalar_add(kgT_sb[:, mi, 0:G], ps_k[:, 0:G],
                                            b_sb["bk"][:, mi:mi + 1])
            ps_vg = psA.tile([128, 512], f32, name="ps_vg", tag="pj")
            for kj in range(3):
                nc.tensor.matmul(ps_vg[0:G, 0:TD], lhsT=tokgT_sb[:, kj, :],
                                 rhs=w_sb["wv"][:, kj, :],
                                 start=kj == 0, stop=kj == 2)
            nc.vector.tensor_copy(
                vg_aug[0:G, :, 0:D],
                ps_vg[0:G, 0:TD].rearrange("p (h d) -> p h d", h=H))
            nc.vector.memset(vg_aug[0:G, :, D:E], 1.0)

            def qg_unit(mi):
                ms = slice(mi * 128, (mi + 1) * 128)
                ps_q = psA.tile([128, 512], f32, name="ps_gq", tag="pj")
                for kj in range(3):
                    nc.tensor.matmul(ps_q[:, 0:G], lhsT=w_sb["wqg"][:, kj, ms],
                                     rhs=tokgT_sb[:, kj, :],
                                     start=kj == 0, stop=kj == 2)
                nc.vector.tensor_scalar_add(qgT_sb[:, mi, :], ps_q[:, 0:G],
                                            b_sb["bqg"][:, mi:mi + 1])

            # v_all / vga_all: token-major, all heads, fp16, +ones column
            # (no bias: host adds bv/bvg after the softmax divide)
            v_all = vp.tile([128, KCH, H, E], f16, name="v_all", tag="v_all")
            vga_all = vp.tile([128, NCH, H, E], f16, name="vga_all",
                              tag="vga_all")
            nc.gpsimd.memset(v_all[:, :, :, D:E], 1.0)
            nc.gpsimd.memset(vga_all[:, :, :, D:E], 1.0)

            def v_unit(c, dst, wname, toff):
                ps = psA.tile([128, 512], f32, name="ps_v", tag="pj")
                for kj in range(3):
                    nc.tensor.matmul(
                        ps[:, 0:TD],
                        lhsT=tokT_sb[:, kj,
                                     toff + c * 128:toff + (c + 1) * 128],
                        rhs=w_sb[wname][:, kj, :],
                        start=kj == 0, stop=kj == 2)
                nc.vector.tensor_copy(
                    dst[:, c, :, 0:D],
                    ps[:, 0:TD].rearrange("p (h d) -> p h d", h=H))

            # per-pair projection tiles + unit closures, interleaved
            # [kT u0, qT u0, kT u1, qT u1, ...] so the consuming band loop
            # can be fed by a quota that stays ahead of reads; kga last.
            def make_proj(j, defer_g=False):
                js = slice(j * 128, (j + 1) * 128)
                qT = pairp.tile([128, QLEN], f16, name=f"qT{j}", tag="qT")
                kT = pairp.tile([128, NTOK], f16, name=f"kT{j}", tag="kT")
                kgaT = kgap.tile([128, QLEN], f16, name=f"kgaT{j}",
                                 tag="kgaT")
                units = {}

                def unit(dst, wname, bname, toff, off, nn_, eng):
                    def run():
                        ps = psA.tile([128, 512], f32, name="ps_p", tag="pj")
                        for kj in range(3):
                            nc.tensor.matmul(
                                ps[:, 0:nn_], lhsT=w_sb[wname][:, kj, js],
                                rhs=tokT_sb[:, kj,
                                            toff + off:toff + off + nn_],
                                start=kj == 0, stop=kj == 2)
                        eng.tensor_scalar_add(
                            dst[:, off:off + nn_], ps[:, 0:nn_],
                            b_sb[bname][:, j:j + 1])
                    return run

                for (key, dst, wname, bname, toff, ntk, eng) in (
                        ("k", kT, "wk", "bk", 0, NTOK, nc.vector),
                        ("q", qT, "wq", "bq", HALO, QLEN, nc.vector),
                        ("g", kgaT, "wkg", "bkg", HALO, QLEN, nc.vector)):
                    for ti in range((ntk + 511) // 512):
                        off = ti * 512
                        nn_ = min(512, ntk - off)
                        units[key, ti] = unit(dst, wname, bname, toff, off,
                                              nn_, eng)
                if defer_g:
                    order = [("k", 0), ("q", 0), ("k", 1), ("q", 1),
                             ("k", 2), ("q", 2), ("k", 3), ("q", 3),
                             ("k", 4), ("q", 4), ("k", 5), ("q", 5),
                             ("k", 6), ("q", 6), ("k", 7), ("q", 7),
                             ("k", 8)]
                else:
                    order = [("k", 0), ("q", 0), ("k", 1), ("q", 1),
                             ("k", 2), ("q", 2), ("k", 3), ("q", 3),
                             ("g", 0), ("g", 1), ("k", 4), ("q", 4),
                             ("g", 2), ("g", 3), ("k", 5), ("q", 5),
                             ("g", 4), ("g", 5), ("k", 6), ("q", 6),
                             ("g", 6), ("g", 7), ("k", 7), ("q", 7),
                             ("k", 8)]
                # minimum fill prefix each band slot kk requires
                # (self-feed, with 2 slots of lookahead slack)
                pos = {key: i for i, key in enumerate(order)}
                raw = [max(pos["k", kk // 4],
                           pos["q", min(kk, NCH - 1) // 4]) + 1
                       for kk in range(KCH)]
                need = [raw[min(kk + 2, KCH - 1)] for kk in range(KCH)]
                gu = [units["g", ti] for ti in range(8)] if defer_g else []
                return qT, kT, kgaT, [units[k] for k in order], need, gu

            def band_loop(j, pair, fills, need, vfills):
                """One head-pair band sweep.  `fills` are emitted under a
                quota that keeps proj(j) units ahead of band reads (per the
                `need` prefix); `vfills` maps slot -> extra closures."""
                qT, kT, kgaT = pair[0], pair[1], pair[2]
                pT_live = {}
                nfill = len(fills)
                emitted = 0

                pvst = {}

                def pv_chunk(ci):
                    # out[q, par*2E + hh*E + e]; chunk pairs share one psO
                    # tile so the copy+DMA fire once per pair
                    par = ci % 2
                    if par == 0:
                        pvst["ps_o"] = psO.tile([128, 4 * E], f32,
                                                name="ps_o", tag="ot")
                    ps_o = pvst["ps_o"]
                    for hh in range(2):
                        h = 2 * j + hh
                        hf = hh * 512
                        eo = par * 2 * E + hh * E
                        nc.tensor.matmul(
                            ps_o[:, eo:eo + E],
                            lhsT=pT_live[ci][:, hf + 256:hf + 384],
                            rhs=v_all[:, ci, h, :],
                            start=True, stop=False)
                        nc.tensor.matmul(
                            ps_o[:, eo:eo + E],
                            lhsT=pT_live[ci + 1][:, hf + 128:hf + 256],
                            rhs=v_all[:, ci + 1, h, :],
                            start=False, stop=False)
                        nc.tensor.matmul(
                            ps_o[:, eo:eo + E],
                            lhsT=pT_live[ci + 2][:, hf + 0:hf + 128],
                            rhs=v_all[:, ci + 2, h, :],
                            start=False, stop=False)
                        nc.tensor.matmul(
                            ps_o[:, eo:eo + E],
                            lhsT=pT_live[ci + 2][:, hf + 384:hf + 512],
                            rhs=vg_aug[:, h, :],
                            start=False, stop=True)
                    if par == 1:
                        ot = outp.tile([128, 4 * E], bf16, name="ot",
                                       tag="ot_sb")
                        nc.vector.tensor_copy(ot, ps_o)
                        nc.sync.dma_start(out=out_d[j, ci // 2], in_=ot)
                    del pT_live[ci]

                for kk in range(KCH):
                    # fill quota: even spread, but never behind band reads
                    quota = max((kk + 3) * nfill // KCH, need[kk])
                    quota = min(quota, nfill)
                    while emitted < quota:
                        fills[emitted]()
                        emitted += 1
                    qlo = max(kk - 2, 0)
                    qhi = min(kk, NCH - 1)
                    nq = qhi - qlo + 1
                    glo = 2 - (kk - qlo)
                    ps_s = psS.tile([128, 1024], f32, name="ps_s", tag="sT")
                    for hh in range(2):
                        hof = hh * 512
                        nc.tensor.matmul(
                            ps_s[:, hof + glo * 128:hof + (glo + nq) * 128],
                            lhsT=kT[hh * 64:hh * 64 + 64,
                                    kk * 128:(kk + 1) * 128],
                            rhs=qT[hh * 64:hh * 64 + 64,
                                   qlo * 128:(qhi + 1) * 128],
                            start=True, stop=True)
                        if kk >= 2:
                            ci = kk - 2
                            nc.tensor.matmul(
                                ps_s[:, hof + 384:hof + 512],
                                lhsT=kgT_sb[hh * 64:hh * 64 + 64, j, :],
                                rhs=qT[hh * 64:hh * 64 + 64,
                                       ci * 128:(ci + 1) * 128],
                                start=True, stop=True)
                    pt = pp.tile([128, 1024], f16, name="pT", tag="pT")
                    nc.scalar.activation(pt, ps_s, AF.Exp)
                    mt = m_sb["m_first"] if kk == 0 else (
                        m_sb["m_last"] if kk == KCH - 1 else m_sb["m_std"])
                    ptv = pt.rearrange("p (hh a b q) -> p hh a b q",
                                       hh=2, a=2, b=2)
                    mtv = mt.rearrange("p (hh a q) -> p hh a q", hh=2, a=2)
                    nc.gpsimd.tensor_mul(ptv[:, :, :, 0, :],
                                         ptv[:, :, :, 0, :], mtv)
                    pT_live[kk] = pt
                    for vf in vfills.get(kk, ()):
                        vf()
                    if kk >= 4:
                        pv_chunk(kk - 4)
                while emitted < nfill:
                    fills[emitted]()
                    emitted += 1
                pv_chunk(NCH - 2)
                pv_chunk(NCH - 1)

            gq_state = {}

            def gq_init(j):
                og_acc = outp.tile([G, 2, E], f32, name="og_acc",
                                   tag="og_acc")
                qg2 = outp.tile([128, 2 * G], f16, name="qg2", tag="qg2")
                nc.vector.memset(qg2, 0.0)
                nc.vector.tensor_copy(qg2[0:64, 0:G], qgT_sb[0:64, j, :])
                nc.vector.tensor_copy(qg2[64:128, G:2 * G],
                                      qgT_sb[64:128, j, :])
                gq_state[j] = (og_acc, qg2)

            def gq_group(j, kgaT, gg):
                og_acc, qg2 = gq_state[j]
                grp = list(range(gg * 10, min(gg * 10 + 10, NCH)))
                ps_sg = psS.tile([128, 1024], f32, name="ps_sg", tag="sT")
                for ii, ci in enumerate(grp):
                    nc.tensor.matmul(
                        ps_sg[:, ii * 100:ii * 100 + 100],
                        lhsT=kgaT[:, ci * 128:(ci + 1) * 128],
                        rhs=qg2, start=True, stop=True)
                pg = pp.tile([128, 1024], f16, name="pg", tag="pT")
                nc.scalar.activation(pg[:, 0:len(grp) * 100],
                                     ps_sg[:, 0:len(grp) * 100], AF.Exp)
                for hh in range(2):
                    h = 2 * j + hh
                    ps_pv = psO.tile([128, 2 * E], f32, name="ps_pv",
                                     tag="ot")
                    for ii, ci in enumerate(grp):
                        nc.tensor.matmul(
                            ps_pv[0:G, 0:E],
                            lhsT=pg[:, ii * 100 + hh * G:
                                    ii * 100 + hh * G + G],
                            rhs=vga_all[:, ci, h, :],
                            start=ii == 0, stop=ii == len(grp) - 1)
                    if gg == 0:
                        nc.vector.tensor_copy(og_acc[:, hh, :],
                                              ps_pv[0:G, 0:E])
                    else:
                        nc.vector.tensor_add(og_acc[:, hh, :],
                                             og_acc[:, hh, :],
                                             ps_pv[0:G, 0:E])
                if gg == 3:
                    for hh in range(2):
                        nc.sync.dma_start(out=og_d[2 * j + hh],
                                          in_=og_acc[:, hh, :])

            # schedule: band0(+proj0, v units) -> band1(+proj1, vga 0-11,
            # qgT) -> band2(+proj2, vga 12-31, all gq groups) -> tiny tail
            p0 = make_proj(0, defer_g=True)
            vf0 = {kk: [lambda c=kk: v_unit(c, v_all, "wv", 0)]
                   for kk in range(KCH)}
            band_loop(0, p0, p0[3], p0[4], vf0)
            p1 = make_proj(1)
            vf1 = {kk: [lambda c=kk: v_unit(c, vga_all, "wvg", HALO)]
                   for kk in range(12)}
            # deferred kga(0) units (only needed by gq0 in band2)
            for i, u in enumerate(p0[5]):
                vf1.setdefault(1 + 2 * i, []).append(u)
            for mi in range(3):
                vf1.setdefault(20 + 4 * mi, []).append(
                    lambda m=mi: qg_unit(m))
            band_loop(1, p1, p1[3], p1[4], vf1)
            p2 = make_proj(2)
            vf2 = {kk: [lambda c=kk + 12: v_unit(c, vga_all, "wvg", HALO)]
                   for kk in range(20)}
            vf2.setdefault(8, []).extend(
                [lambda: gq_init(0), lambda: gq_init(1)])
            for gg, sl in enumerate((10, 13, 18, 23)):
                vf2.setdefault(sl, []).append(
                    lambda g=gg: gq_group(0, p0[2], g))
            for gg, sl in enumerate((11, 15, 19, 25)):
                vf2.setdefault(sl, []).append(
                    lambda g=gg: gq_group(1, p1[2], g))
            vf2.setdefault(14, []).append(lambda: gq_init(2))
            for gg, sl in enumerate((16, 21, 27, 29)):
                vf2.setdefault(sl, []).append(
                    lambda g=gg: gq_group(2, p2[2], g))
            band_loop(2, p2, p2[3], p2[4], vf2)

        if reps > 1:
            loop_ctx.__exit__(None, None, None)

    nc.compile()
    return nc


def _get_exec(reps=1):
    """Build + jit the 8-core PJRT executable once per reps; cache it."""
    key = f"exec{reps}"
    if key in _cache:
        return _cache[key]
    import jax
    from jax.sharding import Mesh, PartitionSpec
    from jax.experimental.shard_map import shard_map
    from concourse import bass2jax
    import concourse.mybir as mybir_

    nc = _build_program(reps=reps)
    _cache[f"ncobj{reps}"] = nc
    bass2jax.install_neuronx_cc_hook()
    partition_name = (nc.partition_id_tensor.name
                      if nc.partition_id_tensor else None)
    in_names, out_names, out_avals, zero_shapes = [], [], [], []
    for alloc in nc.m.functions[0].allocations:
        if not isinstance(alloc, mybir_.MemoryLocationSet):
            continue
        name = alloc.memorylocations[0].name
        if alloc.kind == "ExternalInput":
            if name != partition_name:
                in_names.append(name)
        elif alloc.kind == "ExternalOutput":
            shape = tuple(alloc.tensor_shape)
            dtype = mybir_.dt.np(alloc.dtype)
            out_names.append(name)
            out_avals.append(jax.core.ShapedArray(shape, dtype))
            zero_shapes.append((shape, dtype))
    n_params = len(in_names)
    n_outs = len(out_avals)
    all_names = in_names + out_names
    if partition_name is not None:
        all_names = all_names + [partition_name]

    def _body(*args):
        operands = list(args)
        if partition_name is not None:
            operands.append(bass2jax.partition_id_tensor())
        outs = bass2jax._bass_exec_p.bind(
            *operands,
            out_avals=tuple(out_avals),
            in_names=tuple(all_names),
            out_names=tuple(out_names),
            lowering_input_output_aliases=(),
            sim_require_finite=True,
            sim_require_nnan=True,
            nc=nc,
        )
        return tuple(outs)

    donate = tuple(range(n_params, n_params + n_outs))
    devices = jax.devices()[:NCORES]
    mesh = Mesh(np.asarray(devices), ("core",))
    in_specs = (PartitionSpec("core"),) * (n_params + n_outs)
    out_specs = (PartitionSpec("core"),) * n_outs
    sharded = jax.jit(
        shard_map(_body, mesh=mesh, in_specs=in_specs, out_specs=out_specs,
                  check_rep=False),
        donate_argnums=donate, keep_unused=True)
    _cache[key] = (sharded, in_names, out_names, out_avals, zero_shapes)
    return _cache[key]


def _run(in_maps):
    sharded, in_names, out_names, out_avals, zero_shapes = _get_exec()
    concat_in = [
        np.concatenate([in_maps[c][nm] for c in range(NCORES)], axis=0)
        for nm in in_names]
    zeros = [np.zeros((NCORES * s[0], *s[1:]), dt) for s, dt in zero_shapes]
    out_arrs = sharded(*concat_in, *zeros)
    _cache["bench"] = (concat_in, zero_shapes)
    return [
        {nm: np.asarray(out_arrs[i]).reshape(NCORES, *out_avals[i].shape)[c]
         for i, nm in enumerate(out_names)}
        for c in range(NCORES)]


def bench_single(n=10, reps_list=(1, 3)):
    """Single-core timing: run the same SPMD body on device 0 only."""
    import time
    import jax
    from concourse import bass2jax

    concat_in, zero_shapes = _cache["bench"]
    out = {}
    for reps in reps_list:
        sharded, in_names, out_names, out_avals, zshapes = _get_exec(reps)
        # rebuild a single-device body using the same nc
        key = f"exec1core{reps}"
        if key not in _cache:
            nc = _cache[f"ncobj{reps}"]
            partition_name = (nc.partition_id_tensor.name
                              if nc.partition_id_tensor else None)
            all_names = list(in_names) + list(out_names)
            if partition_name is not None:
                all_names.append(partition_name)

            def _body(*args, _nc=nc, _all=tuple(all_names),
                      _outs=tuple(out_names), _avals=tuple(out_avals),
                      _pn=partition_name):
                operands = list(args)
                if _pn is not None:
                    operands.append(bass2jax.partition_id_tensor())
                return tuple(bass2jax._bass_exec_p.bind(
                    *operands, out_avals=_avals, in_names=_all,
                    out_names=_outs, lowering_input_output_aliases=(),
                    sim_require_finite=True, sim_require_nnan=True, nc=_nc))

            n_params = len(in_names)
            donate = tuple(range(n_params, n_params + len(out_names)))
            _cache[key] = jax.jit(_body, donate_argnums=donate,
                                  keep_unused=True)
        fn = _cache[key]
        dev0 = jax.devices()[0]
        per_core = [jax.device_put(a.reshape(NCORES, a.shape[0] // NCORES,
                                             *a.shape[1:])[0], dev0)
                    for a in concat_in]
        for a in per_core:
            a.block_until_ready()
        times = []
        for _ in range(n):
            zeros = [jax.device_put(np.zeros(s, dt), dev0)
                     for s, dt in zero_shapes]
            for z in zeros:
                z.block_until_ready()
            t0 = time.perf_counter()
            o = fn(*per_core, *zeros)
            for x in o:
                x.block_until_ready()
            times.append(time.perf_counter() - t0)
        out[reps] = times
    return out


def _tokens(x):
    b = x.shape[0]
    t = x.reshape(b, C, NS, WS, NS, WS).transpose(0, 1, 2, 4, 3, 5)
    t = t.reshape(b, C, N, WS * WS).transpose(0, 2, 1, 3)
    return np.ascontiguousarray(t.reshape(b, N, TD))


def _untokens(o):
    b = o.shape[0]
    o = o.reshape(b, NS, NS, C, WS, WS).transpose(0, 3, 1, 4, 2, 5)
    return np.ascontiguousarray(o.reshape(b, C, IMG, IMG))


def _make_masks(s):
    # quad mask [hh, a, q] as [128, 512]; a=0 slot masks g0 (triu, q>=p),
    # a=1 slot masks g2 (tril)
    triu = np.triu(np.ones((W1, W1), np.float16))
    tril = np.tril(np.ones((W1, W1), np.float16))
    zer = np.zeros((W1, W1), np.float16)
    std = np.concatenate([triu, tril, triu, tril], axis=1)
    first = std.copy()
    last = std.copy()
    if s == 0:  # global chunk 0: kk=0's g2 slot invalid
        first[:, 128:256] = zer
        first[:, 384:512] = zer
    if s == SPLITS - 1:  # global chunk 127: kk=33's g0 slot invalid
        last[:, 0:128] = zer
        last[:, 256:384] = zer
    return (np.ascontiguousarray(std), np.ascontiguousarray(first),
            np.ascontiguousarray(last))


def kernel(**inputs):
    x = np.asarray(inputs["x"], dtype=np.float32)
    tokens = _tokens(x)  # (B, N, TD)
    scale = np.float32(1.0 / np.sqrt(D))

    host_w = {
        "wq": np.asarray(inputs["Wq"], np.float32) * scale,
        "wk": np.asarray(inputs["Wk"], np.float32),
        "wv": np.asarray(inputs["Wv"], np.float32),
        "wkg": np.asarray(inputs["Wkg"], np.float32),
        "wvg": np.asarray(inputs["Wvg"], np.float32),
        "wqg": np.asarray(inputs["Wqg"], np.float32) * scale,
    }
    # device layout: [128, 3, TD] (partition-contiguous DMA)
    host_w = {k: np.ascontiguousarray(
        v.astype(np.float16).reshape(3, 128, TD).transpose(1, 0, 2))
        for k, v in host_w.items()}
    host_b = {
        "bq": np.asarray(inputs["bq"], np.float32) * scale,
        "bk": np.asarray(inputs["bk"], np.float32),
        "bkg": np.asarray(inputs["bkg"], np.float32),
        "bqg": np.asarray(inputs["bqg"], np.float32) * scale,
    }
    ball = np.ascontiguousarray(np.concatenate(
        [host_b[k].reshape(3, 128).T for k in ("bq", "bk", "bkg", "bqg")],
        axis=1))
    bv = np.asarray(inputs["bv"], np.float32)       # added on host
    bvg = np.asarray(inputs["bvg"], np.float32)     # added on host

    in_maps = []
    for core in range(NCORES):
        b, s = divmod(core, SPLITS)
        lo = s * QLEN - HALO
        hi = (s + 1) * QLEN + HALO
        shard = np.zeros((NTOK, TD), np.float32)
        s0, s1 = max(lo, 0), min(hi, N)
        shard[s0 - lo:s1 - lo] = tokens[b, s0:s1]
        tokT = np.ascontiguousarray(shard.T.astype(np.float16))
        tokgT = tokens[b, GPOS].T.astype(np.float16)  # (TD, G)
        tokg_p = tokgT.reshape(3, 128, G).transpose(1, 0, 2).reshape(128, -1)
        m_std, m_first, m_last = _make_masks(s)
        small = np.ascontiguousarray(np.concatenate(
            [m_std, m_first, m_last, tokg_p], axis=1).astype(np.float16))
        m = dict(host_w)
        m["ball"] = ball
        m["small"] = small
        m["tokT"] = tokT
        in_maps.append(m)

    results = _run(in_maps)

    out = np.empty((B, N, TD), np.float32)
    og_acc = np.zeros((B, H, G, E), np.float64)
    bv_h = bv.reshape(H, D)
    for core in range(NCORES):
        b, s = divmod(core, SPLITS)
        arr = np.asarray(results[core]["out_t"], np.float32)
        # (H//2, NCH//2, 128, 2, 2, E): [j, pair, q, parity, hh, e]
        arr = arr.reshape(H // 2, NCH // 2, W1, 2, 2, E)
        num = arr[..., 0:D]
        den = arr[..., D:E]
        o = num / den + bv_h.reshape(H // 2, 1, 1, 1, 2, D)
        # token = pair*256 + parity*128 + q; feature = (2j+hh)*64 + d
        o = o.transpose(1, 3, 2, 0, 4, 5).reshape(QLEN, TD)
        out[b, s * QLEN:(s + 1) * QLEN] = o
        og_acc[b] += results[core]["og_part"]
    og = (og_acc[..., :D] / og_acc[..., D:]).astype(np.float32)
    og = og + bvg.reshape(1, H, 1, D)
    og = og.transpose(0, 2, 1, 3).reshape(B, G, TD)  # (B, G, H*D)
    out[:, GPOS] = og
    return _untokens(out)
